# revision 2
# baseline (speedup 1.0000x reference)
"""Self-contained Trainium2 Bass kernel: DeBERTa-style disentangled MHA.

Model (per reference):
    q = x @ Wq.T + bq ; k = x @ Wk.T + bk ; v = x @ Wv.T + bv   (per-head split)
    pos_k = rel_emb @ Wk.T + bk ; pos_q = rel_emb @ Wq.T + bq
    scores[i,j] = (q_i.k_j + A[i, i-j+s] + B[j, i-j+s]) * scale + mask
        where A[i,t] = q_i . pos_k[t],  B[j,t] = k_j . pos_q[t]
    out = softmax_j(scores) @ v

Sharding: 8-way head-parallel (2 heads/core), every core handles all 8 batch rows.
Scores are computed transposed (k index on partitions) so probs feed the PV matmul
directly; the softmax denominator comes from an appended ones-column on V.
The relative-position diagonal gathers ("shear") go through a DRAM round trip
in bf16: 384-wide windows are written with row pitch 384 and read back with
row pitch 383, which turns the per-row relative shift into a plain strided DMA.

The whole kernel is software-pipelined over the batch axis: iteration i emits
projections + bias windows + shear DMAs for batch i interleaved with the
score/softmax/PV phase for batch i-1, so the shear round trip hides under
compute and the PE stays busy (HAM clock stays at 2.4 GHz).
"""

import numpy as np

B, S, DIM, H, HD = 8, 512, 1024, 16, 64
NCORES = 8
HPC = H // NCORES            # heads per core = 2
SCALE = float((HD * 3) ** -0.5)
P = 384                      # shear window pitch per 128-row tile
SEG = P * 128                # flat DRAM segment per (m, I, jh) block

_prog_cache = {}


def _build_program():
    import concourse.bass as bass
    import concourse.mybir as mybir
    import concourse.tile as tile
    from concourse import bacc
    from concourse.masks import make_identity

    BF = mybir.dt.bfloat16
    F32 = mybir.dt.float32
    AO = mybir.AluOpType
    AF = mybir.ActivationFunctionType

    nc = bacc.Bacc(None, target_bir_lowering=False, debug=False)

    def ap_of(t, extra_off, dims):
        return bass.AP(t.tensor, int(t.offset) + extra_off, dims)

    names = {}

    with tile.TileContext(nc) as tc:
        with tc.tile_pool(name="dram", bufs=1, space="DRAM") as dram, \
             tc.tile_pool(name="const", bufs=1) as const, \
             tc.tile_pool(name="persist", bufs=1) as persist, \
             tc.tile_pool(name="work", bufs=1) as work, \
             tc.tile_pool(name="dscratch", bufs=1, space="DRAM") as dscratch, \
             tc.tile_pool(name="ps", bufs=1, space="PSUM") as ps:

            # ---------------- I/O ----------------
            xT_d = dram.tile([DIM, B * S], BF, kind="ExternalInput", name="xT")
            relT_d = dram.tile([DIM, 2 * S], BF, kind="ExternalInput", name="relT")
            wqT_d = dram.tile([DIM, 128], BF, kind="ExternalInput", name="wqT")
            wkT_d = dram.tile([DIM, 128], BF, kind="ExternalInput", name="wkT")
            wvT_d = dram.tile([DIM, 128], BF, kind="ExternalInput", name="wvT")
            bq_d = dram.tile([128], F32, kind="ExternalInput", name="bq")
            bk_d = dram.tile([128], F32, kind="ExternalInput", name="bk")
            bv_d = dram.tile([128], F32, kind="ExternalInput", name="bv")
            mask_d = dram.tile([B, S], F32, kind="ExternalInput", name="mask")
            out_d = dram.tile([B * HPC, HD + 1, S], F32, kind="ExternalOutput",
                              name="out")
            for k, t in [("xT", xT_d), ("relT", relT_d), ("wqT", wqT_d),
                         ("wkT", wkT_d), ("wvT", wvT_d), ("bq", bq_d),
                         ("bk", bk_d), ("bv", bv_d), ("mask", mask_d),
                         ("out", out_d)]:
                names[k] = t.name

            # ---------------- persistent SBUF ----------------
            ident = const.tile([128, 128], BF)
            make_identity(nc, ident)
            bq_sb = const.tile([128, 1], F32)
            bk_sb = const.tile([128, 1], F32)
            bv_sb = const.tile([128, 1], F32)
            nc.sync.dma_start(out=bq_sb, in_=bq_d.rearrange("(p o) -> p o", o=1))
            nc.sync.dma_start(out=bk_sb, in_=bk_d.rearrange("(p o) -> p o", o=1))
            nc.sync.dma_start(out=bv_sb, in_=bv_d.rearrange("(p o) -> p o", o=1))
            # mask_sb[p, b*4+J] = mask[b, 128J + p]
            mask_sb = const.tile([128, B, 4], F32)
            nc.sync.dma_start(
                out=mask_sb,
                in_=ap_of(mask_d, 0, [[1, 128], [S, B], [128, 4]]))

            wq_sb = persist.tile([128, 8, 128], BF)
            wk_sb = persist.tile([128, 8, 128], BF)
            wv_sb = persist.tile([128, 8, 128], BF)
            for wsb, wd in [(wq_sb, wqT_d), (wk_sb, wkT_d), (wv_sb, wvT_d)]:
                nc.sync.dma_start(
                    out=wsb, in_=wd.rearrange("(k p) o -> p k o", p=128))

            QT = persist.tile([128, B * S], BF)       # (x@WqT + bq)*scale, transposed
            KT = persist.tile([128, B * S], BF)       # x@WkT + bk, transposed
            posKTr = persist.tile([128, 2 * S], BF)   # pos_k^T, t-axis reversed
            posQT = persist.tile([128, 2 * S], BF)    # (pos_q^T)*scale
            relch = persist.tile([128, 8, 2 * S], BF)
            posKT_tmp = persist.tile([128, 2 * S], BF)

            def big():
                return ps.tile([128, 512], F32, name="big", tag="big", bufs=3)

            # ---------------- pos projections ----------------
            for k in range(8):
                nc.sync.dma_start(out=relch[:, k],
                                  in_=relT_d[128 * k:128 * k + 128, :])
            for tt in range(2):
                sl = slice(512 * tt, 512 * tt + 512)
                pspk = big()
                pspq = big()
                for k in range(8):
                    fl = dict(start=(k == 0), stop=(k == 7))
                    nc.tensor.matmul(pspk, wk_sb[:, k, :], relch[:, k, sl], **fl)
                    nc.tensor.matmul(pspq, wq_sb[:, k, :], relch[:, k, sl], **fl)
                nc.vector.tensor_scalar_add(posKT_tmp[:, sl], pspk, bk_sb)
                nc.vector.tensor_scalar(posQT[:, sl], pspq, bq_sb, SCALE,
                                        AO.add, AO.mult)
            # reversed copy: posKTr[:, u] = posKT_tmp[:, 1023 - u]
            nc.vector.tensor_copy(
                posKTr,
                ap_of(posKT_tmp, 2 * S - 1, [[2 * S, 128], [-1, 2 * S]]))

            # ---------------- pipelined batch loop ----------------
            # iteration i: stage A (proj+windows+shear DMA) for batch i,
            #              stage B (scores+softmax+PV+out) for batch i-1.
            xsb = {}
            gathC = {}
            gathP = {}
            Vaug = {}

            def prefetch_x(b):
                t = work.tile([128, 8, 512], BF, name="xsb", tag="xsb", bufs=2)
                nc.sync.dma_start(
                    out=t,
                    in_=ap_of(xT_d, 512 * b,
                              [[B * S, 128], [128 * B * S, 8], [1, 512]]))
                xsb[b] = t

            prefetch_x(0)

            for it in range(B + 1):
                if it < B:
                    b = it
                    if b + 1 < B:
                        prefetch_x(b + 1)
                    xb = xsb.pop(b)
                    sl = slice(512 * b, 512 * b + 512)

                    # Q/K projections for this batch's 512 tokens
                    psq = big()
                    for k in range(8):
                        nc.tensor.matmul(psq, wq_sb[:, k, :], xb[:, k, :],
                                         start=(k == 0), stop=(k == 7))
                    nc.vector.tensor_scalar(QT[:, sl], psq, bq_sb, SCALE,
                                            AO.add, AO.mult)
                    psk = big()
                    for k in range(8):
                        nc.tensor.matmul(psk, wk_sb[:, k, :], xb[:, k, :],
                                         start=(k == 0), stop=(k == 7))
                    nc.vector.tensor_scalar_add(KT[:, sl], psk, bk_sb)

                    # A = q . pos_k_rev windows ; B = k . pos_q windows
                    ABsb = {}
                    for h in range(HPC):
                        ABsb[h] = work.tile([128, 16, P], BF, name=f"ABsb{h}",
                                            tag=f"ABsb{h}", bufs=2)
                    for m in range(2):
                        lhs = QT if m == 0 else KT
                        rhs = posKTr if m == 0 else posQT
                        for I in range(4):
                            for jh in range(2):
                                w0 = 384 - 128 * I + 256 * jh
                                seg = 8 * m + 2 * I + jh
                                pt = {}
                                for h in range(HPC):
                                    hp = slice(64 * h, 64 * h + 64)
                                    pt[h] = big()
                                    lw = lhs[hp, 512 * b + 128 * I:
                                             512 * b + 128 * I + 128]
                                    nc.tensor.matmul(pt[h][:, 0:P], lw,
                                                     rhs[hp, w0:w0 + P],
                                                     start=True, stop=True,
                                                     tile_position=(64 * h, 0))
                                nc.vector.tensor_copy(ABsb[0][:, seg],
                                                      pt[0][:, 0:P])
                                nc.scalar.copy(ABsb[1][:, seg], pt[1][:, 0:P])

                    # shear round trip: contiguous write (SWDGE), strided
                    # gather-reads (HWDGE on sync + scalar sequencers)
                    for h in range(HPC):
                        abflat = dscratch.tile([16 * SEG], BF,
                                               name=f"abflat{h}",
                                               tag=f"abflat{h}", bufs=2)
                        nc.gpsimd.dma_start(
                            out=ap_of(abflat, 0, [[P, 128], [SEG, 16], [1, P]]),
                            in_=ABsb[h][:])
                        gC = work.tile([128, 4, 512], BF, name=f"gathC{h}",
                                       tag=f"gathC{h}", bufs=2)
                        nc.sync.dma_start(
                            out=gC,
                            in_=ap_of(abflat, 127,
                                      [[P - 1, 128], [SEG, 8], [1, 256]]))
                        gP = work.tile([128, 4, 512], BF, name=f"gathP{h}",
                                       tag=f"gathP{h}", bufs=2)
                        nc.scalar.dma_start(
                            out=gP,
                            in_=ap_of(abflat, 8 * SEG + 128,
                                      [[P - 1, 128], [SEG, 8], [1, 256]]))
                        gathC[(b, h)] = gC
                        gathP[(b, h)] = gP

                    # V projection + transpose into PV lhsT layout
                    psv = big()
                    for k in range(8):
                        nc.tensor.matmul(psv, wv_sb[:, k, :], xb[:, k, :],
                                         start=(k == 0), stop=(k == 7))
                    vsb = work.tile([128, 512], BF, name="vsb", tag="vsb",
                                    bufs=2)
                    nc.vector.tensor_scalar_add(vsb, psv, bv_sb)
                    pvt = big()
                    for J in range(4):
                        nc.tensor.matmul(pvt[:, 128 * J:128 * J + 128],
                                         vsb[:, 128 * J:128 * J + 128], ident,
                                         start=(J == 0), stop=(J == 3),
                                         skip_group_check=(J > 0))
                    # Vaug[:, J, 65h : 65h+65] = [v rows | ones] for PV lhsT
                    va = work.tile([128, 4, 130], BF, name="Vaug", tag="Vaug",
                                   bufs=2)
                    nc.vector.memset(va[:, :, 64:65], 1.0)
                    nc.vector.memset(va[:, :, 129:130], 1.0)
                    for J in range(4):
                        nc.vector.tensor_copy(va[:, J, 0:64],
                                              pvt[:, 128 * J:128 * J + 64])
                        nc.vector.tensor_copy(va[:, J, 65:129],
                                              pvt[:, 128 * J + 64:128 * J + 128])
                    Vaug[b] = va

                if it >= 1:
                    b = it - 1
                    sl = slice(512 * b, 512 * b + 512)
                    va = Vaug.pop(b)
                    gC = [gathC.pop((b, h)) for h in range(HPC)]
                    gP = [gathP.pop((b, h)) for h in range(HPC)]
                    pvps = {}
                    for h in range(HPC):
                        pvps[h] = ps.tile([65, 512], F32, name=f"pv{h}",
                                          tag=f"pv{h}", bufs=1)
                    for J in range(4):
                        jb = slice(512 * b + 128 * J, 512 * b + 128 * J + 128)
                        qkps = {}
                        for h in range(HPC):
                            hp = slice(64 * h, 64 * h + 64)
                            qkps[h] = ps.tile([128, 512], F32, name=f"qk{h}",
                                              tag="qk", bufs=3)
                            nc.tensor.matmul(qkps[h], KT[hp, jb], QT[hp, sl],
                                             start=True, stop=False,
                                             tile_position=(64 * h, 0))
                        for h in range(HPC):
                            for I in range(4):
                                nc.tensor.matmul(
                                    qkps[h][:, 128 * I:128 * I + 128],
                                    gC[h][:, I, 128 * J:128 * J + 128],
                                    ident, start=False, stop=False,
                                    skip_group_check=True)
                            nc.tensor.matmul(qkps[h], ident, gP[h][:, J, :],
                                             start=False, stop=True)
                        for h in range(HPC):
                            PT = work.tile([128, 512], BF, name=f"PT{h}",
                                           tag=f"PT{h}", bufs=2)
                            nc.scalar.activation(
                                PT, qkps[h], AF.Exp,
                                bias=mask_sb[:, b, J:J + 1], scale=1.0)
                            nc.tensor.matmul(pvps[h],
                                             va[:, J, 65 * h:65 * h + 65],
                                             PT, start=(J == 0), stop=(J == 3))
                    for h in range(HPC):
                        outsb = work.tile([65, 512], F32, name=f"outsb{h}",
                                          tag=f"outsb{h}", bufs=2)
                        nc.vector.tensor_copy(outsb, pvps[h])
                        nc.scalar.dma_start(out=out_d[HPC * b + h], in_=outsb)

    nc.compile()
    return nc, names


def _get_program():
    if "prog" not in _prog_cache:
        _prog_cache["prog"] = _build_program()
    return _prog_cache["prog"]


def _host_prep(x, rel_embeddings, attn_mask, Wq, bq, Wk, bk, Wv, bv):
    import ml_dtypes
    bf = ml_dtypes.bfloat16
    x = np.asarray(x, np.float32)
    xT = np.ascontiguousarray(x.reshape(B * S, DIM).T).astype(bf)
    relT = np.ascontiguousarray(np.asarray(rel_embeddings, np.float32).T).astype(bf)
    WqT = np.asarray(Wq, np.float32).T
    WkT = np.asarray(Wk, np.float32).T
    WvT = np.asarray(Wv, np.float32).T
    mask = np.ascontiguousarray(
        np.asarray(attn_mask, np.float32).reshape(B, S))
    bq = np.asarray(bq, np.float32)
    bk = np.asarray(bk, np.float32)
    bv = np.asarray(bv, np.float32)
    maps = []
    for c in range(NCORES):
        sl = slice(128 * c, 128 * c + 128)
        maps.append({
            "xT": xT,
            "relT": relT,
            "wqT": np.ascontiguousarray(WqT[:, sl]).astype(bf),
            "wkT": np.ascontiguousarray(WkT[:, sl]).astype(bf),
            "wvT": np.ascontiguousarray(WvT[:, sl]).astype(bf),
            "bq": np.ascontiguousarray(bq[sl]),
            "bk": np.ascontiguousarray(bk[sl]),
            "bv": np.ascontiguousarray(bv[sl]),
            "mask": mask,
        })
    return maps


def kernel(x, rel_embeddings, attn_mask, Wq, bq, Wk, bk, Wv, bv):
    from concourse.bass_utils import run_bass_kernel_spmd

    nc, names = _get_program()
    maps = _host_prep(x, rel_embeddings, attn_mask, Wq, bq, Wk, bk, Wv, bv)
    in_maps = [{names[k]: v for k, v in m.items()} for m in maps]
    res = run_bass_kernel_spmd(nc, in_maps, list(range(NCORES)))
    out = np.empty((B, S, DIM), np.float32)
    for c in range(NCORES):
        o = np.asarray(res.results[c][names["out"]], np.float32)
        for b in range(B):
            for hl in range(HPC):
                d0 = 128 * c + 64 * hl
                blk = o[HPC * b + hl]          # [65, 512]: rows 0-63 PV, row 64 L
                out[b, :, d0:d0 + 64] = (blk[0:64] / blk[64:65]).T
    return out


# revision 3
# speedup vs baseline: 1.0781x; 1.0781x over previous
"""Self-contained Trainium2 Bass kernel: DeBERTa-style disentangled MHA.

Model (per reference):
    q = x @ Wq.T + bq ; k = x @ Wk.T + bk ; v = x @ Wv.T + bv   (per-head split)
    pos_k = rel_emb @ Wk.T + bk ; pos_q = rel_emb @ Wq.T + bq
    scores[i,j] = (q_i.k_j + A[i, i-j+s] + B[j, i-j+s]) * scale + mask
        where A[i,t] = q_i . pos_k[t],  B[j,t] = k_j . pos_q[t]
    out = softmax_j(scores) @ v

Sharding: 8-way head-parallel (2 heads/core), every core handles all 8 batch rows.
Scores are computed transposed (k index on partitions) so probs feed the PV matmul
directly; the softmax denominator comes from an appended ones-column on V.
The relative-position diagonal gathers ("shear") go through a DRAM round trip
in bf16: 384-wide windows are written with row pitch 384 and read back with
row pitch 383, which turns the per-row relative shift into a plain strided DMA.

The whole kernel is software-pipelined over the batch axis: iteration i emits
projections + bias windows + shear DMAs for batch i interleaved with the
score/softmax/PV phase for batch i-1, so the shear round trip hides under
compute and the PE stays busy (HAM clock stays at 2.4 GHz).
"""

import numpy as np

B, S, DIM, H, HD = 8, 512, 1024, 16, 64
NCORES = 8
HPC = H // NCORES            # heads per core = 2
SCALE = float((HD * 3) ** -0.5)
P = 384                      # shear window pitch per 128-row tile
SEG = P * 128                # flat DRAM segment per (m, I, jh) block

_prog_cache = {}


def _build_program():
    import concourse.bass as bass
    import concourse.mybir as mybir
    import concourse.tile as tile
    from concourse import bacc
    from concourse.masks import make_identity

    BF = mybir.dt.bfloat16
    F32 = mybir.dt.float32
    AO = mybir.AluOpType
    AF = mybir.ActivationFunctionType

    nc = bacc.Bacc(None, target_bir_lowering=False, debug=False)

    def ap_of(t, extra_off, dims):
        return bass.AP(t.tensor, int(t.offset) + extra_off, dims)

    names = {}

    with tile.TileContext(nc) as tc:
        with tc.tile_pool(name="dram", bufs=1, space="DRAM") as dram, \
             tc.tile_pool(name="const", bufs=1) as const, \
             tc.tile_pool(name="persist", bufs=1) as persist, \
             tc.tile_pool(name="work", bufs=1) as work, \
             tc.tile_pool(name="dscratch", bufs=1, space="DRAM") as dscratch, \
             tc.tile_pool(name="ps", bufs=1, space="PSUM") as ps:

            # ---------------- I/O ----------------
            xT_d = dram.tile([DIM, B * S], BF, kind="ExternalInput", name="xT")
            relT_d = dram.tile([DIM, 2 * S], BF, kind="ExternalInput", name="relT")
            wqT_d = dram.tile([DIM, 128], BF, kind="ExternalInput", name="wqT")
            wkT_d = dram.tile([DIM, 128], BF, kind="ExternalInput", name="wkT")
            wvT_d = dram.tile([DIM, 128], BF, kind="ExternalInput", name="wvT")
            bq_d = dram.tile([128], F32, kind="ExternalInput", name="bq")
            bk_d = dram.tile([128], F32, kind="ExternalInput", name="bk")
            bv_d = dram.tile([128], F32, kind="ExternalInput", name="bv")
            mask_d = dram.tile([B, S], F32, kind="ExternalInput", name="mask")
            out_d = dram.tile([B * HPC, HD + 1, S], F32, kind="ExternalOutput",
                              name="out")
            for k, t in [("xT", xT_d), ("relT", relT_d), ("wqT", wqT_d),
                         ("wkT", wkT_d), ("wvT", wvT_d), ("bq", bq_d),
                         ("bk", bk_d), ("bv", bv_d), ("mask", mask_d),
                         ("out", out_d)]:
                names[k] = t.name

            # ---------------- persistent SBUF ----------------
            ident = const.tile([128, 128], BF)
            make_identity(nc, ident)
            bq_sb = const.tile([128, 1], F32)
            bk_sb = const.tile([128, 1], F32)
            bv_sb = const.tile([128, 1], F32)
            nc.sync.dma_start(out=bq_sb, in_=bq_d.rearrange("(p o) -> p o", o=1))
            nc.sync.dma_start(out=bk_sb, in_=bk_d.rearrange("(p o) -> p o", o=1))
            nc.sync.dma_start(out=bv_sb, in_=bv_d.rearrange("(p o) -> p o", o=1))
            # mask_sb[p, b*4+J] = mask[b, 128J + p]
            mask_sb = const.tile([128, B, 4], F32)
            nc.sync.dma_start(
                out=mask_sb,
                in_=ap_of(mask_d, 0, [[1, 128], [S, B], [128, 4]]))

            wq_sb = persist.tile([128, 8, 128], BF)
            wk_sb = persist.tile([128, 8, 128], BF)
            wv_sb = persist.tile([128, 8, 128], BF)
            for wsb, wd in [(wq_sb, wqT_d), (wk_sb, wkT_d), (wv_sb, wvT_d)]:
                nc.sync.dma_start(
                    out=wsb, in_=wd.rearrange("(k p) o -> p k o", p=128))

            QT = persist.tile([128, B * S], BF)       # (x@WqT + bq)*scale, transposed
            KT = persist.tile([128, B * S], BF)       # x@WkT + bk, transposed
            posKTr = persist.tile([128, 2 * S], BF)   # pos_k^T, t-axis reversed
            posQT = persist.tile([128, 2 * S], BF)    # (pos_q^T)*scale
            relch = persist.tile([128, 8, 2 * S], BF)
            posKT_tmp = persist.tile([128, 2 * S], BF)

            def big():
                return ps.tile([128, 512], F32, name="big", tag="big", bufs=3)

            # ---------------- pos projections ----------------
            for k in range(8):
                nc.sync.dma_start(out=relch[:, k],
                                  in_=relT_d[128 * k:128 * k + 128, :])
            for tt in range(2):
                sl = slice(512 * tt, 512 * tt + 512)
                pspk = big()
                pspq = big()
                for k in range(8):
                    fl = dict(start=(k == 0), stop=(k == 7))
                    nc.tensor.matmul(pspk, wk_sb[:, k, :], relch[:, k, sl], **fl)
                    nc.tensor.matmul(pspq, wq_sb[:, k, :], relch[:, k, sl], **fl)
                nc.vector.tensor_scalar_add(posKT_tmp[:, sl], pspk, bk_sb)
                nc.vector.tensor_scalar(posQT[:, sl], pspq, bq_sb, SCALE,
                                        AO.add, AO.mult)
            # reversed copy: posKTr[:, u] = posKT_tmp[:, 1023 - u]
            nc.vector.tensor_copy(
                posKTr,
                ap_of(posKT_tmp, 2 * S - 1, [[2 * S, 128], [-1, 2 * S]]))

            # ---------------- pipelined batch loop ----------------
            # iteration i: stage A (proj+windows+shear DMA) for batch i,
            #              stage B (scores+softmax+PV+out) for batch i-1.
            xsb = {}
            gathC = {}
            gathP = {}
            Vaug = {}

            def prefetch_x(b):
                t = work.tile([128, 8, 512], BF, name="xsb", tag="xsb", bufs=2)
                nc.sync.dma_start(
                    out=t,
                    in_=ap_of(xT_d, 512 * b,
                              [[B * S, 128], [128 * B * S, 8], [1, 512]]))
                xsb[b] = t

            prefetch_x(0)

            for it in range(B + 1):
                if it < B:
                    b = it
                    if b + 1 < B:
                        prefetch_x(b + 1)
                    xb = xsb.pop(b)
                    sl = slice(512 * b, 512 * b + 512)

                    # Q/K projections for this batch's 512 tokens
                    psq = big()
                    for k in range(8):
                        nc.tensor.matmul(psq, wq_sb[:, k, :], xb[:, k, :],
                                         start=(k == 0), stop=(k == 7))
                    nc.vector.tensor_scalar(QT[:, sl], psq, bq_sb, SCALE,
                                            AO.add, AO.mult)
                    psk = big()
                    for k in range(8):
                        nc.tensor.matmul(psk, wk_sb[:, k, :], xb[:, k, :],
                                         start=(k == 0), stop=(k == 7))
                    nc.vector.tensor_scalar_add(KT[:, sl], psk, bk_sb)

                    # A = q . pos_k_rev windows ; B = k . pos_q windows
                    ABsb = {}
                    for h in range(HPC):
                        ABsb[h] = work.tile([128, 16, P], BF, name=f"ABsb{h}",
                                            tag=f"ABsb{h}", bufs=2)
                    for m in range(2):
                        lhs = QT if m == 0 else KT
                        rhs = posKTr if m == 0 else posQT
                        for I in range(4):
                            for jh in range(2):
                                w0 = 384 - 128 * I + 256 * jh
                                seg = 8 * m + 2 * I + jh
                                pt = {}
                                for h in range(HPC):
                                    hp = slice(64 * h, 64 * h + 64)
                                    pt[h] = big()
                                    lw = lhs[hp, 512 * b + 128 * I:
                                             512 * b + 128 * I + 128]
                                    nc.tensor.matmul(pt[h][:, 0:P], lw,
                                                     rhs[hp, w0:w0 + P],
                                                     start=True, stop=True,
                                                     tile_position=(64 * h, 0))
                                nc.vector.tensor_copy(ABsb[0][:, seg],
                                                      pt[0][:, 0:P])
                                nc.scalar.copy(ABsb[1][:, seg], pt[1][:, 0:P])

                    # shear round trip: contiguous write (SWDGE), strided
                    # gather-reads (HWDGE on sync + scalar sequencers)
                    for h in range(HPC):
                        abflat = dscratch.tile([16 * SEG], BF,
                                               name=f"abflat{h}",
                                               tag=f"abflat{h}", bufs=2)
                        nc.gpsimd.dma_start(
                            out=ap_of(abflat, 0, [[P, 128], [SEG, 16], [1, P]]),
                            in_=ABsb[h][:])
                        gC = work.tile([128, 4, 512], BF, name=f"gathC{h}",
                                       tag=f"gathC{h}", bufs=2)
                        nc.sync.dma_start(
                            out=gC,
                            in_=ap_of(abflat, 127,
                                      [[P - 1, 128], [SEG, 8], [1, 256]]))
                        gP = work.tile([128, 4, 512], BF, name=f"gathP{h}",
                                       tag=f"gathP{h}", bufs=2)
                        nc.sync.dma_start(
                            out=gP,
                            in_=ap_of(abflat, 8 * SEG + 128,
                                      [[P - 1, 128], [SEG, 8], [1, 256]]))
                        gathC[(b, h)] = gC
                        gathP[(b, h)] = gP

                    # V projection + transpose into PV lhsT layout
                    psv = big()
                    for k in range(8):
                        nc.tensor.matmul(psv, wv_sb[:, k, :], xb[:, k, :],
                                         start=(k == 0), stop=(k == 7))
                    vsb = work.tile([128, 512], BF, name="vsb", tag="vsb",
                                    bufs=2)
                    nc.vector.tensor_scalar_add(vsb, psv, bv_sb)
                    pvt = big()
                    for J in range(4):
                        nc.tensor.matmul(pvt[:, 128 * J:128 * J + 128],
                                         vsb[:, 128 * J:128 * J + 128], ident,
                                         start=(J == 0), stop=(J == 3),
                                         skip_group_check=(J > 0))
                    # Vaug[:, J, 65h : 65h+65] = [v rows | ones] for PV lhsT
                    va = work.tile([128, 4, 130], BF, name="Vaug", tag="Vaug",
                                   bufs=2)
                    nc.vector.memset(va[:, :, 64:65], 1.0)
                    nc.vector.memset(va[:, :, 129:130], 1.0)
                    for J in range(4):
                        nc.vector.tensor_copy(va[:, J, 0:64],
                                              pvt[:, 128 * J:128 * J + 64])
                        nc.vector.tensor_copy(va[:, J, 65:129],
                                              pvt[:, 128 * J + 64:128 * J + 128])
                    Vaug[b] = va

                if it >= 1:
                    b = it - 1
                    sl = slice(512 * b, 512 * b + 512)
                    va = Vaug.pop(b)
                    gC = [gathC.pop((b, h)) for h in range(HPC)]
                    gP = [gathP.pop((b, h)) for h in range(HPC)]
                    pvps = {}
                    for h in range(HPC):
                        pvps[h] = ps.tile([65, 512], F32, name=f"pv{h}",
                                          tag=f"pv{h}", bufs=1)
                    for J in range(4):
                        jb = slice(512 * b + 128 * J, 512 * b + 128 * J + 128)
                        qkps = {}
                        for h in range(HPC):
                            hp = slice(64 * h, 64 * h + 64)
                            qkps[h] = ps.tile([128, 512], F32, name=f"qk{h}",
                                              tag="qk", bufs=3)
                            nc.tensor.matmul(qkps[h], KT[hp, jb], QT[hp, sl],
                                             start=True, stop=False,
                                             tile_position=(64 * h, 0))
                        for h in range(HPC):
                            for I in range(4):
                                nc.tensor.matmul(
                                    qkps[h][:, 128 * I:128 * I + 128],
                                    gC[h][:, I, 128 * J:128 * J + 128],
                                    ident, start=False, stop=False,
                                    skip_group_check=True)
                            nc.tensor.matmul(qkps[h], ident, gP[h][:, J, :],
                                             start=False, stop=True)
                        for h in range(HPC):
                            PT = work.tile([128, 512], BF, name=f"PT{h}",
                                           tag=f"PT{h}", bufs=2)
                            nc.scalar.activation(
                                PT, qkps[h], AF.Exp,
                                bias=mask_sb[:, b, J:J + 1], scale=1.0)
                            nc.tensor.matmul(pvps[h],
                                             va[:, J, 65 * h:65 * h + 65],
                                             PT, start=(J == 0), stop=(J == 3))
                    for h in range(HPC):
                        outsb = work.tile([65, 512], F32, name=f"outsb{h}",
                                          tag=f"outsb{h}", bufs=2)
                        nc.vector.tensor_copy(outsb, pvps[h])
                        nc.scalar.dma_start(out=out_d[HPC * b + h], in_=outsb)

    nc.compile()
    return nc, names


def _get_program():
    if "prog" not in _prog_cache:
        _prog_cache["prog"] = _build_program()
    return _prog_cache["prog"]


def _host_prep(x, rel_embeddings, attn_mask, Wq, bq, Wk, bk, Wv, bv):
    import ml_dtypes
    bf = ml_dtypes.bfloat16
    x = np.asarray(x, np.float32)
    xT = np.ascontiguousarray(x.reshape(B * S, DIM).T).astype(bf)
    relT = np.ascontiguousarray(np.asarray(rel_embeddings, np.float32).T).astype(bf)
    WqT = np.asarray(Wq, np.float32).T
    WkT = np.asarray(Wk, np.float32).T
    WvT = np.asarray(Wv, np.float32).T
    mask = np.ascontiguousarray(
        np.asarray(attn_mask, np.float32).reshape(B, S))
    bq = np.asarray(bq, np.float32)
    bk = np.asarray(bk, np.float32)
    bv = np.asarray(bv, np.float32)
    maps = []
    for c in range(NCORES):
        sl = slice(128 * c, 128 * c + 128)
        maps.append({
            "xT": xT,
            "relT": relT,
            "wqT": np.ascontiguousarray(WqT[:, sl]).astype(bf),
            "wkT": np.ascontiguousarray(WkT[:, sl]).astype(bf),
            "wvT": np.ascontiguousarray(WvT[:, sl]).astype(bf),
            "bq": np.ascontiguousarray(bq[sl]),
            "bk": np.ascontiguousarray(bk[sl]),
            "bv": np.ascontiguousarray(bv[sl]),
            "mask": mask,
        })
    return maps


def kernel(x, rel_embeddings, attn_mask, Wq, bq, Wk, bk, Wv, bv):
    from concourse.bass_utils import run_bass_kernel_spmd

    nc, names = _get_program()
    maps = _host_prep(x, rel_embeddings, attn_mask, Wq, bq, Wk, bk, Wv, bv)
    in_maps = [{names[k]: v for k, v in m.items()} for m in maps]
    res = run_bass_kernel_spmd(nc, in_maps, list(range(NCORES)))
    out = np.empty((B, S, DIM), np.float32)
    for c in range(NCORES):
        o = np.asarray(res.results[c][names["out"]], np.float32)
        for b in range(B):
            for hl in range(HPC):
                d0 = 128 * c + 64 * hl
                blk = o[HPC * b + hl]          # [65, 512]: rows 0-63 PV, row 64 L
                out[b, :, d0:d0 + 64] = (blk[0:64] / blk[64:65]).T
    return out


# revision 11
# speedup vs baseline: 1.2496x; 1.1591x over previous
"""Self-contained Trainium2 Bass kernel: DeBERTa-style disentangled MHA.

Model (per reference):
    q = x @ Wq.T + bq ; k = x @ Wk.T + bk ; v = x @ Wv.T + bv   (per-head split)
    pos_k = rel_emb @ Wk.T + bk ; pos_q = rel_emb @ Wq.T + bq
    scores[i,j] = (q_i.k_j + A[i, i-j+s] + B[j, i-j+s]) * scale + mask
        where A[i,t] = q_i . pos_k[t],  B[j,t] = k_j . pos_q[t]
    out = softmax_j(scores) @ v

Sharding: 8-way head-parallel (2 heads/core), every core handles all 8 batch rows.
Scores are computed transposed (k index on partitions) so probs feed the PV matmul
directly; the softmax denominator comes from an appended ones-column on V.
The relative-position diagonal gathers ("shear") go through a DRAM round trip
in bf16: 384-wide windows are written with row pitch 384 and read back with
row pitch 383, which turns the per-row relative shift into a plain strided DMA.

The whole kernel is software-pipelined over the batch axis: iteration i emits
projections + bias windows + shear DMAs for batch i interleaved with the
score/softmax/PV phase for batch i-1, so the shear round trip hides under
compute and the PE stays busy (HAM clock stays at 2.4 GHz).
"""

import numpy as np

B, S, DIM, H, HD = 8, 512, 1024, 16, 64
NCORES = 8
HPC = H // NCORES            # heads per core = 2
SCALE = float((HD * 3) ** -0.5)
P = 384                      # shear window pitch per 128-row tile
SEG = P * 128                # flat DRAM segment per (m, I, jh) block

_prog_cache = {}


def _build_program():
    import concourse.bass as bass
    import concourse.mybir as mybir
    import concourse.tile as tile
    from concourse import bacc
    from concourse.masks import make_identity

    BF = mybir.dt.bfloat16
    F32 = mybir.dt.float32
    AO = mybir.AluOpType
    AF = mybir.ActivationFunctionType

    nc = bacc.Bacc(None, target_bir_lowering=False, debug=False)

    def ap_of(t, extra_off, dims):
        return bass.AP(t.tensor, int(t.offset) + extra_off, dims)

    names = {}

    with tile.TileContext(nc) as tc:
        with tc.tile_pool(name="dram", bufs=1, space="DRAM") as dram, \
             tc.tile_pool(name="const", bufs=1) as const, \
             tc.tile_pool(name="persist", bufs=1) as persist, \
             tc.tile_pool(name="work", bufs=1) as work, \
             tc.tile_pool(name="dscratch", bufs=1, space="DRAM") as dscratch, \
             tc.tile_pool(name="ps", bufs=1, space="PSUM") as ps:

            # ---------------- I/O ----------------
            xT_d = dram.tile([DIM, B * S], BF, kind="ExternalInput", name="xT")
            relT_d = dram.tile([DIM, 2 * S], BF, kind="ExternalInput", name="relT")
            wqT_d = dram.tile([DIM, 128], BF, kind="ExternalInput", name="wqT")
            wkT_d = dram.tile([DIM, 128], BF, kind="ExternalInput", name="wkT")
            wvT_d = dram.tile([DIM, 128], BF, kind="ExternalInput", name="wvT")
            bq_d = dram.tile([128], F32, kind="ExternalInput", name="bq")
            bk_d = dram.tile([128], F32, kind="ExternalInput", name="bk")
            bv_d = dram.tile([128], F32, kind="ExternalInput", name="bv")
            mask_d = dram.tile([B, S], F32, kind="ExternalInput", name="mask")
            out_d = dram.tile([B * HPC, HD + 1, S], F32, kind="ExternalOutput",
                              name="out")
            for k, t in [("xT", xT_d), ("relT", relT_d), ("wqT", wqT_d),
                         ("wkT", wkT_d), ("wvT", wvT_d), ("bq", bq_d),
                         ("bk", bk_d), ("bv", bv_d), ("mask", mask_d),
                         ("out", out_d)]:
                names[k] = t.name

            # ---------------- persistent SBUF ----------------
            ident = const.tile([128, 128], BF)
            make_identity(nc, ident)
            bq_sb = const.tile([128, 1], F32)
            bk_sb = const.tile([128, 1], F32)
            bv_sb = const.tile([128, 1], F32)
            nc.sync.dma_start(out=bq_sb, in_=bq_d.rearrange("(p o) -> p o", o=1))
            nc.sync.dma_start(out=bk_sb, in_=bk_d.rearrange("(p o) -> p o", o=1))
            nc.sync.dma_start(out=bv_sb, in_=bv_d.rearrange("(p o) -> p o", o=1))
            # mask_sb[p, b*4+J] = mask[b, 128J + p]
            mask_sb = const.tile([128, B, 4], F32)
            nc.sync.dma_start(
                out=mask_sb,
                in_=ap_of(mask_d, 0, [[1, 128], [S, B], [128, 4]]))

            wq_sb = persist.tile([128, 8, 128], BF)
            wk_sb = persist.tile([128, 8, 128], BF)
            wv_sb = persist.tile([128, 8, 128], BF)
            for wsb, wd in [(wq_sb, wqT_d), (wk_sb, wkT_d), (wv_sb, wvT_d)]:
                nc.sync.dma_start(
                    out=wsb, in_=wd.rearrange("(k p) o -> p k o", p=128))

            QT = persist.tile([128, B * S], BF)       # (x@WqT + bq)*scale, transposed
            KT = persist.tile([128, B * S], BF)       # x@WkT + bk, transposed
            posKTr = persist.tile([128, 2 * S], BF)   # pos_k^T, t-axis reversed
            posQT = persist.tile([128, 2 * S], BF)    # (pos_q^T)*scale
            relch = persist.tile([128, 8, 2 * S], BF)
            posKT_tmp = persist.tile([128, 2 * S], BF)

            def big():
                return ps.tile([128, 512], F32, name="big", tag="big", bufs=3)

            # ---------------- pos projections ----------------
            for k in range(8):
                nc.scalar.dma_start(out=relch[:, k],
                                    in_=relT_d[128 * k:128 * k + 128, :])
            for tt in range(2):
                sl = slice(512 * tt, 512 * tt + 512)
                pspk = big()
                pspq = big()
                for k in range(8):
                    fl = dict(start=(k == 0), stop=(k == 7))
                    nc.tensor.matmul(pspk, wk_sb[:, k, :], relch[:, k, sl], **fl)
                    nc.tensor.matmul(pspq, wq_sb[:, k, :], relch[:, k, sl], **fl)
                nc.vector.tensor_scalar_add(posKT_tmp[:, sl], pspk, bk_sb)
                nc.vector.tensor_scalar(posQT[:, sl], pspq, bq_sb, SCALE,
                                        AO.add, AO.mult)
            # reversed copy: posKTr[:, u] = posKT_tmp[:, 1023 - u]
            nc.vector.tensor_copy(
                posKTr,
                ap_of(posKT_tmp, 2 * S - 1, [[2 * S, 128], [-1, 2 * S]]))

            # ---------------- pipelined batch loop ----------------
            # iteration i: stage A (proj+windows+shear DMA) for batch i,
            #              stage B (scores+softmax+PV+out) for batch i-1.
            xsb = {}
            gathC = {}
            gathP = {}
            Vaug = {}

            def prefetch_x(b):
                t = work.tile([128, 8, 512], BF, name="xsb", tag="xsb", bufs=2)
                nc.sync.dma_start(
                    out=t,
                    in_=ap_of(xT_d, 512 * b,
                              [[B * S, 128], [128 * B * S, 8], [1, 512]]))
                xsb[b] = t

            prefetch_x(0)

            for it in range(B + 2):
                if it < B:
                    b = it
                    if b + 1 < B:
                        prefetch_x(b + 1)
                    xb = xsb.pop(b)
                    sl = slice(512 * b, 512 * b + 512)

                    # Q/K projections for this batch's 512 tokens
                    psq = big()
                    for k in range(8):
                        nc.tensor.matmul(psq, wq_sb[:, k, :], xb[:, k, :],
                                         start=(k == 0), stop=(k == 7))
                    nc.vector.tensor_scalar(QT[:, sl], psq, bq_sb, SCALE,
                                            AO.add, AO.mult)
                    psk = big()
                    for k in range(8):
                        nc.tensor.matmul(psk, wk_sb[:, k, :], xb[:, k, :],
                                         start=(k == 0), stop=(k == 7))
                    nc.vector.tensor_scalar_add(KT[:, sl], psk, bk_sb)

                    # A = q . pos_k_rev windows ; B = k . pos_q windows
                    ABsb = {}
                    for h in range(HPC):
                        ABsb[h] = work.tile([128, 16, P], BF, name=f"ABsb{h}",
                                            tag=f"ABsb{h}", bufs=3)
                    for m in range(2):
                        lhs = QT if m == 0 else KT
                        rhs = posKTr if m == 0 else posQT
                        for I in range(4):
                            for jh in range(2):
                                w0 = 384 - 128 * I + 256 * jh
                                seg = 8 * m + 2 * I + jh
                                pt = {}
                                for h in range(HPC):
                                    hp = slice(64 * h, 64 * h + 64)
                                    pt[h] = big()
                                    lw = lhs[hp, 512 * b + 128 * I:
                                             512 * b + 128 * I + 128]
                                    nc.tensor.matmul(pt[h][:, 0:P], lw,
                                                     rhs[hp, w0:w0 + P],
                                                     start=True, stop=True,
                                                     tile_position=(64 * h, 0))
                                nc.vector.tensor_copy(ABsb[0][:, seg],
                                                      pt[0][:, 0:P])
                                nc.scalar.copy(ABsb[1][:, seg], pt[1][:, 0:P])

                    # shear round trip: contiguous write (SWDGE), strided
                    # gather-reads (HWDGE on sync + scalar sequencers)
                    for h in range(HPC):
                        abflat = dscratch.tile([16 * SEG], BF,
                                               name=f"abflat{h}",
                                               tag=f"abflat{h}", bufs=3)
                        nc.gpsimd.dma_start(
                            out=ap_of(abflat, 0, [[P, 128], [SEG, 16], [1, P]]),
                            in_=ABsb[h][:])
                        gC = work.tile([128, 4, 512], BF, name=f"gathC{h}",
                                       tag=f"gathC{h}", bufs=3)
                        nc.sync.dma_start(
                            out=gC,
                            in_=ap_of(abflat, 127,
                                      [[P - 1, 128], [SEG, 8], [1, 256]]))
                        gP = work.tile([128, 4, 512], BF, name=f"gathP{h}",
                                       tag=f"gathP{h}", bufs=3)
                        nc.sync.dma_start(
                            out=gP,
                            in_=ap_of(abflat, 8 * SEG + 128,
                                      [[P - 1, 128], [SEG, 8], [1, 256]]))
                        gathC[(b, h)] = gC
                        gathP[(b, h)] = gP

                    # V projection + transpose into PV lhsT layout
                    psv = big()
                    for k in range(8):
                        nc.tensor.matmul(psv, wv_sb[:, k, :], xb[:, k, :],
                                         start=(k == 0), stop=(k == 7))
                    vsb = work.tile([128, 512], BF, name="vsb", tag="vsb",
                                    bufs=2)
                    nc.vector.tensor_scalar_add(vsb, psv, bv_sb)
                    pvt = big()
                    for J in range(4):
                        nc.tensor.matmul(pvt[:, 128 * J:128 * J + 128],
                                         vsb[:, 128 * J:128 * J + 128], ident,
                                         start=(J == 0), stop=(J == 3),
                                         skip_group_check=(J > 0))
                    # Vaug[:, J, 65h : 65h+65] = [v rows | ones] for PV lhsT
                    va = work.tile([128, 4, 130], BF, name="Vaug", tag="Vaug",
                                   bufs=3)
                    nc.vector.memset(va[:, :, 64:65], 1.0)
                    nc.vector.memset(va[:, :, 129:130], 1.0)
                    for J in range(4):
                        nc.vector.tensor_copy(va[:, J, 0:64],
                                              pvt[:, 128 * J:128 * J + 64])
                        nc.vector.tensor_copy(va[:, J, 65:129],
                                              pvt[:, 128 * J + 64:128 * J + 128])
                    Vaug[b] = va

                if it >= 2:
                    b = it - 2
                    sl = slice(512 * b, 512 * b + 512)
                    va = Vaug.pop(b)
                    gC = [gathC.pop((b, h)) for h in range(HPC)]
                    gP = [gathP.pop((b, h)) for h in range(HPC)]
                    pvps = {}
                    for h in range(HPC):
                        pvps[h] = ps.tile([65, 512], F32, name=f"pv{h}",
                                          tag=f"pv{h}", bufs=1)
                    for J in range(4):
                        jb = slice(512 * b + 128 * J, 512 * b + 128 * J + 128)
                        qkps = {}
                        for h in range(HPC):
                            hp = slice(64 * h, 64 * h + 64)
                            qkps[h] = ps.tile([128, 512], F32, name=f"qk{h}",
                                              tag="qk", bufs=3)
                            nc.tensor.matmul(qkps[h], KT[hp, jb], QT[hp, sl],
                                             start=True, stop=False,
                                             tile_position=(64 * h, 0))
                        for h in range(HPC):
                            for I in range(4):
                                nc.tensor.matmul(
                                    qkps[h][:, 128 * I:128 * I + 128],
                                    gC[h][:, I, 128 * J:128 * J + 128],
                                    ident, start=False, stop=False,
                                    skip_group_check=True)
                            nc.tensor.matmul(qkps[h], ident, gP[h][:, J, :],
                                             start=False, stop=True)
                        for h in range(HPC):
                            PT = work.tile([128, 512], BF, name=f"PT{h}",
                                           tag=f"PT{h}", bufs=2)
                            nc.scalar.activation(
                                PT, qkps[h], AF.Exp,
                                bias=mask_sb[:, b, J:J + 1], scale=1.0)
                            nc.tensor.matmul(pvps[h],
                                             va[:, J, 65 * h:65 * h + 65],
                                             PT, start=(J == 0), stop=(J == 3))
                    for h in range(HPC):
                        outsb = work.tile([65, 512], F32, name=f"outsb{h}",
                                          tag=f"outsb{h}", bufs=2)
                        nc.vector.tensor_copy(outsb, pvps[h])
                        nc.scalar.dma_start(out=out_d[HPC * b + h], in_=outsb)

    nc.compile()
    return nc, names


def _get_program():
    if "prog" not in _prog_cache:
        _prog_cache["prog"] = _build_program()
    return _prog_cache["prog"]


def _host_prep(x, rel_embeddings, attn_mask, Wq, bq, Wk, bk, Wv, bv):
    import ml_dtypes
    bf = ml_dtypes.bfloat16
    x = np.asarray(x, np.float32)
    xT = np.ascontiguousarray(x.reshape(B * S, DIM).T).astype(bf)
    relT = np.ascontiguousarray(np.asarray(rel_embeddings, np.float32).T).astype(bf)
    WqT = np.asarray(Wq, np.float32).T
    WkT = np.asarray(Wk, np.float32).T
    WvT = np.asarray(Wv, np.float32).T
    mask = np.ascontiguousarray(
        np.asarray(attn_mask, np.float32).reshape(B, S))
    bq = np.asarray(bq, np.float32)
    bk = np.asarray(bk, np.float32)
    bv = np.asarray(bv, np.float32)
    maps = []
    for c in range(NCORES):
        sl = slice(128 * c, 128 * c + 128)
        maps.append({
            "xT": xT,
            "relT": relT,
            "wqT": np.ascontiguousarray(WqT[:, sl]).astype(bf),
            "wkT": np.ascontiguousarray(WkT[:, sl]).astype(bf),
            "wvT": np.ascontiguousarray(WvT[:, sl]).astype(bf),
            "bq": np.ascontiguousarray(bq[sl]),
            "bk": np.ascontiguousarray(bk[sl]),
            "bv": np.ascontiguousarray(bv[sl]),
            "mask": mask,
        })
    return maps


def kernel(x, rel_embeddings, attn_mask, Wq, bq, Wk, bk, Wv, bv):
    from concourse.bass_utils import run_bass_kernel_spmd

    nc, names = _get_program()
    maps = _host_prep(x, rel_embeddings, attn_mask, Wq, bq, Wk, bk, Wv, bv)
    in_maps = [{names[k]: v for k, v in m.items()} for m in maps]
    res = run_bass_kernel_spmd(nc, in_maps, list(range(NCORES)))
    out = np.empty((B, S, DIM), np.float32)
    for c in range(NCORES):
        o = np.asarray(res.results[c][names["out"]], np.float32)
        for b in range(B):
            for hl in range(HPC):
                d0 = 128 * c + 64 * hl
                blk = o[HPC * b + hl]          # [65, 512]: rows 0-63 PV, row 64 L
                out[b, :, d0:d0 + 64] = (blk[0:64] / blk[64:65]).T
    return out
